# revision 7
# baseline (speedup 1.0000x reference)
"""Trainium2 Bass kernel for nn_NeuralMemoryManager (Titans-style test-time update).

128 independent "bots", each: one fwd/bwd/AdamW(step=1) update of a per-bot
MLP 512->1024->512 (exact GELU), grad-norm clip, shared value-target proj.

Sharding: bot axis B=128 split across 8 NeuronCores (16 bots/core), pure data
parallelism, no collectives.

Key structure exploited:
  - grads are rank-1: gW1 = outer(x, dh_pre), gW2 = outer(h, d_pred)
  - AdamW step-1 collapses to p_new = 0.9999*p - 0.01*g'/(|g'|+1e-8)
  - gnorm^2 = |dhp|^2(1+|x|^2) + |dp|^2(1+|h|^2)  (small-vector norms only)
  - wchange^2 = sum((pn-p)^2) ~= sum(S'^2) (+1e-7 rel), S' = lr*g'/(|g'|+eps)
Engines: PE builds G=u(x)d and D=100|u|(x)|d|+1e-6 (fp32r, consistent so the
saturated s=g/(|g|+eps) is insensitive to the rounding), DVE does 1/D and
G*R, ScalarE does squares/gelu (one act-table set), GpSimd the in-place
weight combine.
"""
import sys

sys.path.insert(0, "/opt/trn_rl_repo")

import numpy as np
from contextlib import ExitStack

import concourse.bass as bass
import concourse.mybir as mybir
import concourse.tile as tile
from concourse import bacc
from concourse.bass_utils import run_bass_kernel_spmd
from concourse.masks import make_identity

F32 = mybir.dt.float32
F32R = mybir.dt.float32r
U32 = mybir.dt.uint32
AF = mybir.ActivationFunctionType
ALU = mybir.AluOpType
AX = mybir.AxisListType

N_CORES = 8
B_FULL, DIM, HID = 128, 512, 1024
BPC = B_FULL // N_CORES  # bots per core = 16
LR, WD = 0.01, 0.01
A_DECAY = 1.0 - LR * WD  # 0.9999
EPS_G = 1e-8             # AdamW eps (after lr-folding: denom 100|g|+1e-6)


def _act(nc, out, in_, func, bias=0.0, scale=1.0, accum_out=None):
    """InstActivation emitter; out = func(in_*scale + bias), accum = row sum."""
    eng = nc.scalar
    inputs = [eng.lower_ap(in_)]
    for arg in (bias, scale, 0.0):
        if isinstance(arg, bass.AP):
            inputs.append(eng.lower_ap(arg))
        else:
            inputs.append(mybir.ImmediateValue(dtype=F32, value=arg))
    outputs = [eng.lower_ap(out)]
    if accum_out is not None:
        outputs.append(eng.lower_ap(accum_out))
    return eng.add_instruction(
        mybir.InstActivation(
            name=nc.get_next_instruction_name(), func=func, ins=inputs, outs=outputs
        )
    )


def _emit_rsqrt(nc, out, a, tmp_pool):
    """out = 1/sqrt(a) on [1,1] f32 tiles via bit-trick + 3 Newton iters.

    Avoids the ScalarE Sqrt activation table (keeps ACT on one table set).
    Seed via float-domain magic (int ALU ops run float semantics on DVE):
    y0_bits = round(0x5f3759df - 0.5*float(a_bits)).
    """
    bf = tmp_pool.tile([1, 1], F32, tag="rsq_bf")
    nc.vector.tensor_copy(bf[:], a[:].bitcast(U32))      # u32 -> f32 convert
    y0f = tmp_pool.tile([1, 1], F32, tag="rsq_y0f")
    nc.vector.tensor_scalar(out=y0f[:], in0=bf[:], scalar1=-0.5,
                            scalar2=float(0x5F3759DF), op0=ALU.mult, op1=ALU.add)
    y = tmp_pool.tile([1, 1], F32, tag="rsq_y")
    nc.vector.tensor_copy(y[:].bitcast(U32), y0f[:])     # f32 -> u32 convert
    for _ in range(3):
        y2 = tmp_pool.tile([1, 1], F32, tag="rsq_y2")
        nc.vector.tensor_tensor(out=y2[:], in0=y[:], in1=y[:], op=ALU.mult)
        ay2 = tmp_pool.tile([1, 1], F32, tag="rsq_ay2")
        nc.vector.tensor_tensor(out=ay2[:], in0=a[:], in1=y2[:], op=ALU.mult)
        f = tmp_pool.tile([1, 1], F32, tag="rsq_f")
        nc.vector.tensor_scalar(
            out=f[:], in0=ay2[:], scalar1=-0.5, scalar2=1.5, op0=ALU.mult, op1=ALU.add
        )
        yn = tmp_pool.tile([1, 1], F32, tag="rsq_y")
        nc.vector.tensor_tensor(out=yn[:], in0=y[:], in1=f[:], op=ALU.mult)
        y = yn
    nc.vector.tensor_copy(out[:], y[:])


def build_module(passes=1):
    nc = bacc.Bacc("TRN2", target_bir_lowering=False, debug=False,
                   num_devices=N_CORES)

    # ---- DRAM I/O (per-core shard: 16 bots) ----
    x_in = nc.dram_tensor("x", [BPC, DIM], F32, kind="ExternalInput").ap()
    W1_in = nc.dram_tensor("W1", [BPC, DIM, HID], F32, kind="ExternalInput").ap()
    b1_in = nc.dram_tensor("b1", [BPC, HID], F32, kind="ExternalInput").ap()
    W2_in = nc.dram_tensor("W2", [BPC, HID, DIM], F32, kind="ExternalInput").ap()
    b2_in = nc.dram_tensor("b2", [BPC, DIM], F32, kind="ExternalInput").ap()
    Wv_in = nc.dram_tensor("Wv", [DIM, DIM], F32, kind="ExternalInput").ap()
    bv_in = nc.dram_tensor("bv", [DIM], F32, kind="ExternalInput").ap()

    ret_out = nc.dram_tensor("retrieved", [BPC, DIM], F32, kind="ExternalOutput").ap()
    loss_out = nc.dram_tensor("loss", [BPC], F32, kind="ExternalOutput").ap()
    wch_out = nc.dram_tensor("wchange", [BPC], F32, kind="ExternalOutput").ap()
    W1n_out = nc.dram_tensor("W1n", [BPC, DIM, HID], F32, kind="ExternalOutput").ap()
    b1n_out = nc.dram_tensor("b1n", [BPC, HID], F32, kind="ExternalOutput").ap()
    W2n_out = nc.dram_tensor("W2n", [BPC, HID, DIM], F32, kind="ExternalOutput").ap()
    b2n_out = nc.dram_tensor("b2n", [BPC, DIM], F32, kind="ExternalOutput").ap()

    with tile.TileContext(nc) as tc, ExitStack() as ctx:
        # ---- pools ----
        singles = ctx.enter_context(tc.tile_pool(name="singles", bufs=1))
        wpool = ctx.enter_context(tc.tile_pool(name="wpool", bufs=2))     # big W tiles
        vecs = ctx.enter_context(tc.tile_pool(name="vecs", bufs=2))       # per-bot vectors
        scr = ctx.enter_context(tc.tile_pool(name="scr", bufs=3))         # [128,512] scratch
        tiny = ctx.enter_context(tc.tile_pool(name="tiny", bufs=4))       # [1,1]-ish
        pG = ctx.enter_context(tc.tile_pool(name="pG", bufs=2, space="PSUM"))
        pD = ctx.enter_context(tc.tile_pool(name="pD", bufs=2, space="PSUM"))
        pBC = ctx.enter_context(tc.tile_pool(name="pBC", bufs=1, space="PSUM"))
        pF = ctx.enter_context(tc.tile_pool(name="pF", bufs=2, space="PSUM"))
        pRow = ctx.enter_context(tc.tile_pool(name="pRow", bufs=1, space="PSUM"))

        # ---- per-core constants ----
        ident = singles.tile([128, 128], F32)
        make_identity(nc, ident[:])
        ones128r = singles.tile([1, 128], F32R)
        nc.vector.memset(ones128r[:].bitcast(F32), 1.0)
        onesHr = singles.tile([1, HID], F32R)
        nc.vector.memset(onesHr[:].bitcast(F32), 1.0)
        epsHr = singles.tile([1, HID], F32R)
        nc.vector.memset(epsHr[:].bitcast(F32), 1e-6)
        ones128f = singles.tile([1, 128], F32)
        nc.vector.memset(ones128f[:], 1.0)

        # Wv tiles [128,512] x4 (i-chunks), bv cols [128,4]
        Wv_t = []
        for c in range(4):
            wv_c = singles.tile([128, DIM], F32, tag=f"wv{c}")
            nc.sync.dma_start(out=wv_c[:], in_=Wv_in[c * 128:(c + 1) * 128, :])
            Wv_t.append(wv_c)
        bvT = singles.tile([128, 4], F32)
        nc.sync.dma_start(out=bvT[:], in_=bv_in.rearrange("(c p) -> p c", p=128))

        # per-bot loss/wchange collectors
        loss_vec = singles.tile([1, BPC], F32)
        wch_vec = singles.tile([1, BPC], F32)

        for _pass in range(passes):
          for b in range(BPC):
            # ================= loads =================
            A = []  # W1 tiles [128, HID] x4  (i-chunks)
            for c in range(4):
                a_c = wpool.tile([128, HID], F32, tag=f"A{c}")
                nc.sync.dma_start(out=a_c[:], in_=W1_in[b, c * 128:(c + 1) * 128, :])
                A.append(a_c)
            Bt = []  # W2 tiles [128, DIM] x8  (j-chunks)
            for j in range(8):
                b_j = wpool.tile([128, DIM], F32, tag=f"B{j}")
                nc.sync.dma_start(out=b_j[:], in_=W2_in[b, j * 128:(j + 1) * 128, :])
                Bt.append(b_j)
            xr = vecs.tile([1, DIM], F32)
            nc.sync.dma_start(out=xr[:], in_=x_in[b:b + 1, :])
            xc = vecs.tile([128, 4], F32)
            nc.sync.dma_start(out=xc[:], in_=x_in[b].rearrange("(c p) -> p c", p=128))
            b1T = vecs.tile([128, 8], F32)
            nc.sync.dma_start(out=b1T[:], in_=b1_in[b].rearrange("(c p) -> p c", p=128))
            b2T = vecs.tile([128, 4], F32)
            nc.sync.dma_start(out=b2T[:], in_=b2_in[b].rearrange("(c p) -> p c", p=128))

            # ================= forward (fp32 exact) =================
            hpreT = pF.tile([128, 8], F32, tag="fwd")
            for jc in range(8):
                for c in range(4):
                    nc.tensor.matmul(
                        hpreT[:, jc:jc + 1],
                        A[c][:, jc * 128:(jc + 1) * 128],
                        xc[:, c:c + 1],
                        start=(c == 0), stop=(c == 3),
                    )
            z = vecs.tile([128, 8], F32)
            nc.vector.tensor_tensor(out=z[:], in0=hpreT[:], in1=b1T[:], op=ALU.add)
            h = vecs.tile([128, 8], F32)
            _act(nc, h[:], z[:], AF.Gelu)
            gp = vecs.tile([128, 8], F32)
            _act(nc, gp[:], z[:], AF.Derivative_Gelu)

            vT = pF.tile([128, 4], F32, tag="fwd")
            for kc in range(4):
                for c in range(4):
                    nc.tensor.matmul(
                        vT[:, kc:kc + 1],
                        Wv_t[c][:, kc * 128:(kc + 1) * 128],
                        xc[:, c:c + 1],
                        start=(c == 0), stop=(c == 3),
                    )
            v_sb = vecs.tile([128, 4], F32)
            nc.vector.tensor_tensor(out=v_sb[:], in0=vT[:], in1=bvT[:], op=ALU.add)

            predT = pF.tile([128, 4], F32, tag="fwd")
            for kc in range(4):
                for jc in range(8):
                    nc.tensor.matmul(
                        predT[:, kc:kc + 1],
                        Bt[jc][:, kc * 128:(kc + 1) * 128],
                        h[:, jc:jc + 1],
                        start=(jc == 0), stop=(jc == 7),
                    )
            pred = vecs.tile([128, 4], F32)
            nc.vector.tensor_tensor(out=pred[:], in0=predT[:], in1=b2T[:], op=ALU.add)
            nc.sync.dma_start(
                out=ret_out[b].rearrange("(c p) -> p c", p=128), in_=pred[:]
            )

            diff = vecs.tile([128, 4], F32)
            nc.vector.tensor_tensor(out=diff[:], in0=pred[:], in1=v_sb[:], op=ALU.subtract)

            # accumulators: ||x||^2, ||h||^2, sum(diff^2), later ndh
            nx_c = vecs.tile([128, 1], F32)
            sq4 = vecs.tile([128, 4], F32, tag="sq4")
            _act(nc, sq4[:], xc[:], AF.Square, accum_out=nx_c[:])
            nh_c = vecs.tile([128, 1], F32)
            sq8 = vecs.tile([128, 8], F32, tag="sq8")
            _act(nc, sq8[:], h[:], AF.Square, accum_out=nh_c[:])
            nl_c = vecs.tile([128, 1], F32)
            sq4b = vecs.tile([128, 4], F32, tag="sq4")
            _act(nc, sq4b[:], diff[:], AF.Square, accum_out=nl_c[:])

            dp_c = vecs.tile([128, 4], F32)  # d_pred cols = diff * 2/512
            _act(nc, dp_c[:], diff[:], AF.Copy, scale=2.0 / DIM)

            # d_pred row (f32r) + |d_pred| row
            dprow_ps = pRow.tile([1, DIM], F32, tag="prow")
            for c in range(4):
                nc.tensor.matmul(
                    dprow_ps[0:1, c * 128:(c + 1) * 128], dp_c[:, c:c + 1], ident[:]
                )
            dp_row32 = vecs.tile([1, DIM], F32)
            _act(nc, dp_row32[:], dprow_ps[:], AF.Copy)
            dp_row = vecs.tile([1, DIM], F32R)
            _act(nc, dp_row[:], dp_row32[:], AF.Copy)
            da2 = vecs.tile([2, DIM], F32R)  # [ |d_pred| ; ones ]
            _act(nc, da2[0:1, :], dp_row[:].bitcast(F32), AF.Abs)
            nc.sync.dma_start(out=da2[1:2, :].bitcast(F32),
                              in_=onesHr[0:1, 0:DIM].bitcast(F32))

            # d_pred broadcast [128, DIM] on PE in full fp32 (sign-exact dh),
            # then dh via fused stt-accum
            bc_ps = pBC.tile([128, DIM], F32)
            nc.tensor.matmul(bc_ps[:], ones128f[:], dp_row32[:])
            dh8 = vecs.tile([128, 8], F32)
            for j in range(8):
                dhscrap = scr.tile([128, DIM], F32, tag="dhscrap")
                nc.vector.scalar_tensor_tensor(
                    out=dhscrap[:], in0=Bt[j][:], scalar=1.0, in1=bc_ps[:],
                    op0=ALU.mult, op1=ALU.mult, accum_out=dh8[:, j:j + 1],
                )
            dhp = vecs.tile([128, 8], F32)  # dh_pre cols
            nc.vector.tensor_tensor(out=dhp[:], in0=dh8[:], in1=gp[:], op=ALU.mult)
            ndh_c = vecs.tile([128, 1], F32)
            sq8b = vecs.tile([128, 8], F32, tag="sq8")
            _act(nc, sq8b[:], dhp[:], AF.Square, accum_out=ndh_c[:])

            # dh_pre row (f32r) + |dh_pre| row
            dhrow_ps0 = pRow.tile([1, DIM], F32, tag="prow")
            dhrow_ps1 = pRow.tile([1, DIM], F32, tag="prow")
            for j in range(8):
                tgt = dhrow_ps0 if j < 4 else dhrow_ps1
                nc.tensor.matmul(
                    tgt[0:1, (j % 4) * 128:(j % 4 + 1) * 128], dhp[:, j:j + 1], ident[:]
                )
            d_row = vecs.tile([1, HID], F32R)
            _act(nc, d_row[0:1, 0:DIM], dhrow_ps0[:], AF.Copy)
            _act(nc, d_row[0:1, DIM:HID], dhrow_ps1[:], AF.Copy)
            da1 = vecs.tile([2, HID], F32R)  # [ |dh_pre| ; ones ]
            _act(nc, da1[0:1, :], d_row[:].bitcast(F32), AF.Abs)
            nc.sync.dma_start(out=da1[1:2, :].bitcast(F32), in_=onesHr[:].bitcast(F32))

            # h row (for u2 = coef*h)
            hrow_ps0 = pRow.tile([1, DIM], F32, tag="prow")
            hrow_ps1 = pRow.tile([1, DIM], F32, tag="prow")
            for j in range(8):
                tgt = hrow_ps0 if j < 4 else hrow_ps1
                nc.tensor.matmul(
                    tgt[0:1, (j % 4) * 128:(j % 4 + 1) * 128], h[:, j:j + 1], ident[:]
                )

            # ============ gnorm / coef (scalars) ============
            nx1 = tiny.tile([1, 1], F32, tag="nx1")
            nc.gpsimd.tensor_reduce(out=nx1[:], in_=nx_c[:], axis=AX.C, op=ALU.add)
            nh1 = tiny.tile([1, 1], F32, tag="nh1")
            nc.gpsimd.tensor_reduce(out=nh1[:], in_=nh_c[:], axis=AX.C, op=ALU.add)
            nl1 = tiny.tile([1, 1], F32, tag="nl1")
            nc.gpsimd.tensor_reduce(out=nl1[:], in_=nl_c[:], axis=AX.C, op=ALU.add)
            ndh1 = tiny.tile([1, 1], F32, tag="ndh1")
            nc.gpsimd.tensor_reduce(out=ndh1[:], in_=ndh_c[:], axis=AX.C, op=ALU.add)

            # loss output = nl1 / 512
            _act(nc, loss_vec[0:1, b:b + 1], nl1[:], AF.Copy, scale=1.0 / DIM)

            t1 = tiny.tile([1, 1], F32, tag="t1")
            nc.vector.tensor_scalar(out=t1[:], in0=nx1[:], scalar1=1.0, scalar2=1.0,
                                    op0=ALU.mult, op1=ALU.add)
            t2 = tiny.tile([1, 1], F32, tag="t2")
            nc.vector.tensor_tensor(out=t2[:], in0=ndh1[:], in1=t1[:], op=ALU.mult)
            t3 = tiny.tile([1, 1], F32, tag="t3")
            nc.vector.tensor_scalar(out=t3[:], in0=nh1[:], scalar1=1.0, scalar2=1.0,
                                    op0=ALU.mult, op1=ALU.add)
            t4 = tiny.tile([1, 1], F32, tag="t4")
            nc.vector.tensor_tensor(out=t4[:], in0=nl1[:], in1=t3[:], op=ALU.mult)
            t4s = tiny.tile([1, 1], F32, tag="t4s")
            nc.vector.tensor_scalar(out=t4s[:], in0=t4[:],
                                    scalar1=(2.0 / DIM) * (2.0 / DIM), scalar2=0.0,
                                    op0=ALU.mult, op1=ALU.add)
            g2 = tiny.tile([1, 1], F32, tag="g2")
            nc.vector.tensor_tensor(out=g2[:], in0=t2[:], in1=t4s[:], op=ALU.add)
            rs = tiny.tile([1, 1], F32, tag="rs")
            _emit_rsqrt(nc, rs, g2, tiny)
            coef = tiny.tile([1, 1], F32, tag="coef")
            nc.vector.tensor_scalar(out=coef[:], in0=rs[:], scalar1=1.0, scalar2=1.0,
                                    op0=ALU.mult, op1=ALU.min)

            coef_ps = pF.tile([128, 1], F32, tag="fwd")
            nc.tensor.matmul(coef_ps[:], ones128f[:], coef[:])
            coef128 = vecs.tile([128, 1], F32)
            nc.vector.tensor_copy(coef128[:], coef_ps[:])

            # u rows (f32r): u1 = coef*x, u2 = coef*h ; |100*u| rows
            u1_row = vecs.tile([1, DIM], F32R)
            _act(nc, u1_row[:], xr[:], AF.Copy, scale=coef[0:1, 0:1])
            ua1 = vecs.tile([2, DIM], F32R)
            _act(nc, ua1[0:1, :], u1_row[:].bitcast(F32), AF.Abs, scale=100.0)
            nc.sync.dma_start(out=ua1[1:2, :].bitcast(F32),
                              in_=epsHr[0:1, 0:DIM].bitcast(F32))
            u2_row = vecs.tile([1, HID], F32R)
            _act(nc, u2_row[0:1, 0:DIM], hrow_ps0[:], AF.Copy, scale=coef[0:1, 0:1])
            _act(nc, u2_row[0:1, DIM:HID], hrow_ps1[:], AF.Copy, scale=coef[0:1, 0:1])
            ua2 = vecs.tile([2, HID], F32R)
            _act(nc, ua2[0:1, :], u2_row[:].bitcast(F32), AF.Abs, scale=100.0)
            nc.sync.dma_start(out=ua2[1:2, :].bitcast(F32), in_=epsHr[:].bitcast(F32))

            # wchange accumulator [128, 18]
            wacc = vecs.tile([128, 18], F32)

            # ============ b1 / b2 updates (cols) ============
            for (bt, cols, g_src, slot) in ((b1T, 8, dhp, 16), (b2T, 4, dp_c, 17)):
                gcol = vecs.tile([128, cols], F32, tag=f"gcol{slot}")
                nc.vector.tensor_scalar(out=gcol[:], in0=g_src[:],
                                        scalar1=coef128[:, 0:1], scalar2=0.0,
                                        op0=ALU.mult, op1=ALU.add)
                acol = vecs.tile([128, cols], F32, tag=f"acol{slot}")
                _act(nc, acol[:], gcol[:], AF.Abs, scale=100.0)
                dencol = vecs.tile([128, cols], F32, tag=f"dencol{slot}")
                nc.vector.tensor_scalar(out=dencol[:], in0=acol[:],
                                        scalar1=1.0, scalar2=1e-6,
                                        op0=ALU.mult, op1=ALU.add)
                rcol = vecs.tile([128, cols], F32, tag=f"rcol{slot}")
                nc.vector.reciprocal(out=rcol[:], in_=dencol[:])
                scol = vecs.tile([128, cols], F32, tag=f"scol{slot}")
                nc.vector.tensor_tensor(out=scol[:], in0=gcol[:], in1=rcol[:],
                                        op=ALU.mult)
                sqcol = vecs.tile([128, cols], F32, tag=f"sqcol{slot}")
                _act(nc, sqcol[:], scol[:], AF.Square, accum_out=wacc[:, slot:slot + 1])
                bn = vecs.tile([128, cols], F32, tag=f"bn{slot}")
                nc.vector.scalar_tensor_tensor(
                    out=bn[:], in0=bt[:], scalar=A_DECAY, in1=scol[:],
                    op0=ALU.mult, op1=ALU.subtract,
                )
                dst = b1n_out if slot == 16 else b2n_out
                nc.sync.dma_start(
                    out=dst[b].rearrange("(c p) -> p c", p=128), in_=bn[:]
                )

            # ============ big W tiles: 8x W1 halves + 8x W2 ============
            for t in range(16):
                if t < 8:
                    c, half = t // 2, t % 2
                    w_ap = A[c][:, half * DIM:(half + 1) * DIM]
                    lg = u1_row[0:1, c * 128:(c + 1) * 128]
                    rg = d_row[0:1, half * DIM:(half + 1) * DIM]
                    la = ua1[:, c * 128:(c + 1) * 128]
                    ra = da1[:, half * DIM:(half + 1) * DIM]
                else:
                    j = t - 8
                    w_ap = Bt[j][:]
                    lg = u2_row[0:1, j * 128:(j + 1) * 128]
                    rg = dp_row[:]
                    la = ua2[:, j * 128:(j + 1) * 128]
                    ra = da2[:]

                G_ps = pG.tile([128, DIM], F32, tag="G")
                nc.tensor.matmul(G_ps[:], lg, rg)
                D_ps = pD.tile([128, DIM], F32, tag="D")
                nc.tensor.matmul(D_ps[:], la, ra)
                R_sb = scr.tile([128, DIM], F32, tag="R")
                nc.vector.reciprocal(out=R_sb[:], in_=D_ps[:])
                S_sb = scr.tile([128, DIM], F32, tag="S")
                nc.vector.tensor_tensor(out=S_sb[:], in0=G_ps[:], in1=R_sb[:],
                                        op=ALU.mult)
                sqscrap = scr.tile([128, DIM], F32, tag="sqscrap")
                _act(nc, sqscrap[:], S_sb[:], AF.Square, accum_out=wacc[:, t:t + 1])
                # in-place: w = 0.9999*w (ACT), then w -= S (GpSimd), DMA out
                _act(nc, w_ap, w_ap, AF.Copy, scale=A_DECAY)
                nc.gpsimd.tensor_tensor(out=w_ap, in0=w_ap, in1=S_sb[:],
                                        op=ALU.subtract)
                if t < 8:
                    nc.sync.dma_start(
                        out=W1n_out[b, c * 128:(c + 1) * 128,
                                    half * DIM:(half + 1) * DIM],
                        in_=w_ap,
                    )
                else:
                    nc.sync.dma_start(
                        out=W2n_out[b, j * 128:(j + 1) * 128, :], in_=w_ap
                    )

            # ============ wchange ============
            wsum_c = vecs.tile([128, 1], F32)
            nc.vector.tensor_reduce(out=wsum_c[:], in_=wacc[:], axis=AX.X, op=ALU.add)
            w2 = tiny.tile([1, 1], F32, tag="w2")
            nc.gpsimd.tensor_reduce(out=w2[:], in_=wsum_c[:], axis=AX.C, op=ALU.add)
            wrs = tiny.tile([1, 1], F32, tag="wrs")
            _emit_rsqrt(nc, wrs, w2, tiny)
            nc.vector.tensor_tensor(out=wch_vec[0:1, b:b + 1], in0=w2[:], in1=wrs[:],
                                    op=ALU.mult)

        nc.sync.dma_start(out=loss_out.rearrange("(o b) -> o b", o=1), in_=loss_vec[:])
        nc.sync.dma_start(out=wch_out.rearrange("(o b) -> o b", o=1), in_=wch_vec[:])

    nc.compile()
    return nc


_NC_CACHE = None


def kernel(x, W1, b1, W2, b2, Wv, bv):
    global _NC_CACHE
    if _NC_CACHE is None:
        _NC_CACHE = build_module()
    nc = _NC_CACHE

    x = np.ascontiguousarray(x, np.float32)
    W1 = np.ascontiguousarray(W1, np.float32)
    b1 = np.ascontiguousarray(b1, np.float32)
    W2 = np.ascontiguousarray(W2, np.float32)
    b2 = np.ascontiguousarray(b2, np.float32)
    Wv = np.ascontiguousarray(Wv, np.float32)
    bv = np.ascontiguousarray(bv, np.float32)

    in_maps = []
    for i in range(N_CORES):
        s = slice(i * BPC, (i + 1) * BPC)
        in_maps.append({
            "x": x[s], "W1": W1[s], "b1": b1[s], "W2": W2[s], "b2": b2[s],
            "Wv": Wv, "bv": bv,
        })
    res = run_bass_kernel_spmd(nc, in_maps, core_ids=list(range(N_CORES)))
    rs = res.results
    cat = lambda k: np.concatenate([np.asarray(rs[i][k]) for i in range(N_CORES)], 0)
    return (cat("retrieved"), cat("loss"), cat("wchange"),
            cat("W1n"), cat("b1n"), cat("W2n"), cat("b2n"))


if __name__ == "__main__":
    rng = np.random.default_rng(0)
    inputs = {
        "x": rng.standard_normal((B_FULL, DIM)).astype(np.float32),
        "W1": (rng.standard_normal((B_FULL, DIM, HID)) / np.sqrt(DIM)).astype(np.float32),
        "b1": np.zeros((B_FULL, HID), np.float32),
        "W2": (rng.standard_normal((B_FULL, HID, DIM)) / np.sqrt(HID)).astype(np.float32),
        "b2": np.zeros((B_FULL, DIM), np.float32),
        "Wv": (rng.standard_normal((DIM, DIM)) / np.sqrt(DIM)).astype(np.float32),
        "bv": np.zeros((DIM,), np.float32),
    }
    outs = kernel(**inputs)
    for o in outs:
        print(o.shape, o.dtype)
